# revision 18
# baseline (speedup 1.0000x reference)
"""Trainium2 Bass kernel for GPT-2 style attention block (B=2, S=2048, D=1024, H=16).

Sharding (8 cores): data-parallel over batch (2) x tensor-parallel over heads (4 per
core). Each core: QKV projection for its 4 heads over the full sequence, full-seq
causal attention for two head PAIRS (scores matmuls K=64 row-tiled so both heads of
a pair run concurrently in the PE array; softmax reduction folded into the PV matmul
via a ones-column in V), then an
AllToAll over all 8 cores converts head-sharding to sequence-sharding so c_proj runs
reduction-free (two-phase accumulation so the second collective overlaps c_proj's
first half). The batch-select is folded into the sender-side normalize so receivers
just add the two AllToAll halves. Host only concatenates per-core outputs.

Compute dtype bf16 (fp32 PSUM accumulation); fp8 was tried for the PV path and
rejected: post-softmax quantization noise lands ~3.6% relative on the output.
"""
import sys
sys.path.insert(0, '/opt/trn_rl_repo')

import numpy as np
import ml_dtypes

import concourse.bass as bass
import concourse.mybir as mybir
import concourse.tile as tile
from concourse import bacc
from concourse.bass_utils import run_bass_kernel_spmd

B, S, D = 2, 2048, 1024
H, HD = 16, 64
NCORES = 8
HPC = H // 4          # heads per core = 4

F32 = mybir.dt.float32
BF16 = mybir.dt.bfloat16
FP8 = mybir.dt.float8e4
DR = mybir.MatmulPerfMode.DoubleRow
ADD = mybir.AluOpType.add
MULT = mybir.AluOpType.mult
BYPASS = mybir.AluOpType.bypass
EXP = mybir.ActivationFunctionType.Exp


def _emit(nc, tc):
    xT = nc.dram_tensor("xT", [D, S], BF16, kind="ExternalInput").ap()
    w_qk = nc.dram_tensor("w_qk", [D, 512], BF16, kind="ExternalInput").ap()
    w_v = nc.dram_tensor("w_v", [D, 256], BF16, kind="ExternalInput").ap()
    w_p = nc.dram_tensor("w_p", [D, D], BF16, kind="ExternalInput").ap()
    bqk = nc.dram_tensor("bqk", [128, 4], F32, kind="ExternalInput").ap()
    zsel = nc.dram_tensor("zsel", [128, 2], F32, kind="ExternalInput").ap()
    beff = nc.dram_tensor("beff", [128, D], F32, kind="ExternalInput").ap()
    cmask = nc.dram_tensor("cmask", [128, 256], BF16, kind="ExternalInput").ap()
    out = nc.dram_tensor("out", [512, D], F32, kind="ExternalOutput").ap()

    a2a_in = [nc.dram_tensor(f"a2a_in{u}", [8, 128, 512], BF16) for u in range(2)]
    a2a_out = [nc.dram_tensor(f"a2a_out{u}", [8, 128, 512], BF16) for u in range(2)]

    from contextlib import ExitStack
    ctx = ExitStack()
    cst = ctx.enter_context(tc.tile_pool(name="cst", bufs=1))
    pw = ctx.enter_context(tc.tile_pool(name="pw", bufs=4, space="PSUM"))
    psc = ctx.enter_context(tc.tile_pool(name="psc", bufs=2, space="PSUM"))
    sb = ctx.enter_context(tc.tile_pool(name="sb", bufs=3))
    sb2 = ctx.enter_context(tc.tile_pool(name="sb2", bufs=2))

    # ---- constants + PE/ACT warm-up (no DMA deps) ----
    junk = cst.tile([128, 512], BF16)
    nc.vector.memset(junk[:], 0.125)
    jexp = cst.tile([128, 8], F32)
    nc.scalar.activation(out=jexp[:], in_=junk[:, 0:8], func=EXP)  # exp table preload
    ones_sb = cst.tile([1, 64], BF16)
    nc.vector.memset(ones_sb[:], 1.0)
    # full-K warm-up matmuls: real array activity so HAM unthrottles early
    for w in range(8):
        warm_ps = pw.tile([128, 512], F32, tag="w", name=f"warm{w}")
        nc.tensor.matmul(warm_ps[:], junk[:, 0:128], junk[:],
                         start=True, stop=True)

    # ---- resident SBUF loads, ordered so qk_proj can start ASAP ----
    xT_sb = cst.tile([128, 8, S], BF16)
    wqk_sb = cst.tile([128, 8, 512], BF16)
    for k in range(8):
        nc.sync.dma_start(wqk_sb[:, k], w_qk.rearrange("(k p) n -> p k n", p=128)[:, k])
        nc.sync.dma_start(xT_sb[:, k], xT.rearrange("(k p) n -> p k n", p=128)[:, k])
    wv_sb = cst.tile([128, 8, 256], BF16)
    nc.sync.dma_start(wv_sb[:], w_v.rearrange("(k p) n -> p k n", p=128))
    bqk_sb = cst.tile([128, 4], F32)
    nc.sync.dma_start(bqk_sb[:], bqk)
    zsel_sb = cst.tile([128, 2], F32)
    nc.sync.dma_start(zsel_sb[:], zsel)
    cm_sb = cst.tile([128, 256], BF16)
    nc.sync.dma_start(cm_sb[:], cmask)
    beff_sb = cst.tile([128, D], F32)
    nc.sync.dma_start(beff_sb[:], beff)
    wp_sb = cst.tile([128, 8, D], BF16)
    nc.sync.dma_start(wp_sb[:], w_p.rearrange("(k p) n -> p k n", p=128))

    # qkT [512, 2048]: slices 0/1 = q^T pairs 0/1 (2 heads x 64 each, q prescaled
    # 1/8), slices 2/3 = k^T pairs 0/1. Head pair layout: even head partitions
    # 0-63, odd head partitions 64-127.
    qkT_sb = cst.tile([128, 4, S], BF16)

    def qk_proj(m):
        ps = {qt: pw.tile([128, 512], F32, tag="w", name=f"qk{m}_{qt}")
              for qt in range(4)}
        for k in range(8):
            for qt in range(4):
                nc.tensor.matmul(
                    ps[qt][:], wqk_sb[:, k, m * 128:(m + 1) * 128],
                    xT_sb[:, k, qt * 512:(qt + 1) * 512],
                    start=(k == 0), stop=(k == 7))
        for qt in range(4):
            nc.vector.tensor_scalar(
                out=qkT_sb[:, m, qt * 512:(qt + 1) * 512], in0=ps[qt][:],
                scalar1=bqk_sb[:, m:m + 1], scalar2=None, op0=ADD)

    # V with interleaved ones column: V_sb [128, 16, 4*65]
    V_sb = cst.tile([128, 16, HPC * 65], BF16)

    def v_ones():
        nc.vector.memset(
            V_sb[:].rearrange("p m (h c) -> p m h c", c=65)[:, :, :, 64:65], 1.0)

    def v_piece(m):
        ps = pw.tile([128, 512], F32, tag="w", name=f"v{m}")
        for k in range(8):
            nc.tensor.matmul(
                ps[:, :256], xT_sb[:, k, m * 128:(m + 1) * 128], wv_sb[:, k, :],
                start=(k == 0), stop=(k == 7))
        nc.vector.tensor_copy(
            out=V_sb[:, m].rearrange("p (h c) -> p h c", c=65)[:, :, 0:64],
            in_=ps[:, :256].rearrange("p (h c) -> p h c", c=64))

    proj_sb = cst.tile([128, 8, 512], BF16)  # gathered attnT for my 512 q rows
    acc_sb = cst.tile([128, 4, D], F32)      # c_proj phase-0 partials

    def attend_pair(u, qt):
        # heads A=2u (partitions 0-63), B=2u+1 (partitions 64-127)
        hA, hB = 2 * u, 2 * u + 1
        nkb = 4 * qt + 4
        atA = pw.tile([128, 512], F32, tag="w", name=f"atA{u}_{qt}")
        atB = pw.tile([128, 512], F32, tag="w", name=f"atB{u}_{qt}")
        for kb in range(nkb):
            rel = max(0, kb * 128 - qt * 512)
            sc = psc.tile([128, 1024], F32, tag="sc", name=f"sc{kb}")
            # scores^T pair: K=64 row-tiles (0,0) and (64,0) run concurrently
            nc.tensor.matmul(
                sc[:, rel:512],
                qkT_sb[0:64, 2 + u, kb * 128:(kb + 1) * 128],
                qkT_sb[0:64, u, qt * 512 + rel:(qt + 1) * 512],
                start=True, stop=True)
            nc.tensor.matmul(
                sc[:, 512 + rel:1024],
                qkT_sb[64:128, 2 + u, kb * 128:(kb + 1) * 128],
                qkT_sb[64:128, u, qt * 512 + rel:(qt + 1) * 512],
                start=True, stop=True)
            pt = sb.tile([128, 1024], BF16, tag="pt", name=f"pt{kb}")
            sc3 = sc[:].rearrange("p (g c) -> p g c", g=2)[:, :, rel:512]
            pt3 = pt[:].rearrange("p (g c) -> p g c", g=2)[:, :, rel:512]
            nc.scalar.activation(out=pt3, in_=sc3, func=EXP)
            if kb >= 4 * qt:  # diagonal block: zero the masked triangle
                ptm = pt[:].rearrange("p (g c) -> p g c", g=2)[:, :, rel:rel + 128]
                nc.vector.tensor_tensor(
                    ptm, ptm, cm_sb[:].rearrange("p (g c) -> p g c", g=2), MULT)
            nc.tensor.matmul(
                atA[0:65, rel:512], V_sb[:, kb, hA * 65:(hA + 1) * 65],
                pt[:, rel:512], start=(kb == 0), stop=(kb == nkb - 1))
            nc.tensor.matmul(
                atB[0:65, rel:512], V_sb[:, kb, hB * 65:(hB + 1) * 65],
                pt[:, 512 + rel:1024], start=(kb == 0), stop=(kb == nkb - 1))
        # unnormalized attn pair -> SBUF, one reciprocal-broadcast for both heads,
        # then batch-select (zsel) folded into the normalize: two send variants
        stg = sb2.tile([128, 512], BF16, tag="stg")
        nc.vector.tensor_copy(out=stg[0:64, :], in_=atA[0:64, :])
        nc.vector.tensor_copy(out=stg[64:128, :], in_=atB[0:64, :])
        den2 = sb2.tile([1, 1024], F32, tag="den2")
        nc.vector.tensor_copy(out=den2[0:1, 0:512], in_=atA[64:65, :])
        nc.vector.tensor_copy(out=den2[0:1, 512:1024], in_=atB[64:65, :])
        rec2 = sb2.tile([1, 1024], F32, tag="rec2")
        nc.vector.reciprocal_approx_fast(rec2[:], den2[:])
        rec2b = sb2.tile([1, 1024], BF16, tag="rec2b")
        nc.vector.tensor_copy(out=rec2b[:], in_=rec2[:])
        bc = pw.tile([128, 512], F32, tag="w", name=f"bc{u}_{qt}")
        nc.tensor.matmul(bc[0:64, :], ones_sb[:], rec2b[0:1, 0:512],
                         start=True, stop=True)
        nc.tensor.matmul(bc[64:128, :], ones_sb[:], rec2b[0:1, 512:1024],
                         start=True, stop=True)
        for half, tag in ((0, "sA"), (1, "sB")):
            snd = sb2.tile([128, 512], BF16, tag=tag)
            nc.vector.scalar_tensor_tensor(
                out=snd[:], in0=stg[:], scalar=zsel_sb[:, half:half + 1],
                in1=bc[:], op0=MULT, op1=MULT)
            nc.sync.dma_start(a2a_in[u].ap()[4 * half + qt], snd[:])

    def a2a_go(u):
        nc.gpsimd.collective_compute(
            "AllToAll", BYPASS, replica_groups=[list(range(NCORES))],
            ins=[a2a_in[u].ap().opt()], outs=[a2a_out[u].ap().opt()])

    def a2a_recv(u):
        t0 = sb2.tile([128, 4, 512], BF16, tag="t0")
        t1 = sb2.tile([128, 4, 512], BF16, tag="t1")
        nc.sync.dma_start(t0[:], a2a_out[u].ap()[0:4].rearrange("g p n -> p g n"))
        nc.sync.dma_start(t1[:], a2a_out[u].ap()[4:8].rearrange("g p n -> p g n"))
        dst = proj_sb[:].rearrange("p (g t) n -> p g t n", t=2)[:, :, u, :]
        nc.vector.tensor_tensor(dst, t0[:], t1[:], ADD)

    def c_proj_phase0(ms):
        # unit-0 chunks (k = 0,2,4,6) -> acc_sb (+ beff), frees PSUM per step
        for m in ms:
            for n in range(2):
                ps = pw.tile([128, 512], F32, tag="w", name=f"p0_{m}_{n}")
                for ki, k in enumerate((0, 2, 4, 6)):
                    nc.tensor.matmul(
                        ps[:], proj_sb[:, k, m * 128:(m + 1) * 128],
                        wp_sb[:, k, n * 512:(n + 1) * 512],
                        start=(ki == 0), stop=(ki == 3))
                nc.vector.tensor_tensor(
                    acc_sb[:, m, n * 512:(n + 1) * 512], ps[:],
                    beff_sb[:, n * 512:(n + 1) * 512], ADD)

    def c_proj_phase1(ms):
        for m in ms:
            out_sb = sb2.tile([128, D], F32, tag="out")
            for n in range(2):
                ps = pw.tile([128, 512], F32, tag="w", name=f"p1_{m}_{n}")
                for ki, k in enumerate((1, 3, 5, 7)):
                    nc.tensor.matmul(
                        ps[:], proj_sb[:, k, m * 128:(m + 1) * 128],
                        wp_sb[:, k, n * 512:(n + 1) * 512],
                        start=(ki == 0), stop=(ki == 3))
                nc.vector.tensor_tensor(
                    out_sb[:, n * 512:(n + 1) * 512], ps[:],
                    acc_sb[:, m, n * 512:(n + 1) * 512], ADD)
            nc.sync.dma_start(out[m * 128:(m + 1) * 128, :], out_sb[:])

    # ---- emission order tuned for overlap ----
    qk_proj(0)
    qk_proj(2)
    v_ones()
    for m in range(4):
        v_piece(m)
    attend_pair(0, 0)
    for m in range(4, 8):
        v_piece(m)
    attend_pair(0, 1)
    for m in range(8, 12):
        v_piece(m)
    attend_pair(0, 2)
    for m in range(12, 16):
        v_piece(m)
    attend_pair(0, 3)
    a2a_go(0)
    qk_proj(1)
    qk_proj(3)
    attend_pair(1, 0)
    a2a_recv(0)
    attend_pair(1, 1)
    c_proj_phase0((0, 1))
    attend_pair(1, 2)
    c_proj_phase0((2, 3))
    attend_pair(1, 3)
    a2a_go(1)
    for w in range(24):
        warm_ps = pw.tile([128, 512], F32, tag="w", name=f"bridge{w}")
        nc.tensor.matmul(warm_ps[:], junk[:, 0:128], junk[:],
                         start=True, stop=True)
    a2a_recv(1)
    c_proj_phase1((0, 1, 2, 3))

    ctx.close()


def build_nc():
    nc = bacc.Bacc("TRN2", target_bir_lowering=False, debug=False, num_devices=NCORES)
    with tile.TileContext(nc) as tc:
        _emit(nc, tc)
    nc.compile()
    return nc


def shard_inputs(hidden_states, c_attn_w, c_attn_b, c_proj_w, c_proj_b):
    x = np.asarray(hidden_states, np.float32)
    W = np.asarray(c_attn_w, np.float32)
    bqkv = np.asarray(c_attn_b, np.float32)
    Wp = np.asarray(c_proj_w, np.float32)
    bp = np.asarray(c_proj_b, np.float32)

    wq, wk, wv = W[:, :D] * 0.125, W[:, D:2 * D], W[:, 2 * D:]
    bq, bk, bv = bqkv[:D] * 0.125, bqkv[D:2 * D], bqkv[2 * D:]
    beff = np.broadcast_to(bp + bv @ Wp, (128, D)).astype(np.float32).copy()
    wp_bf = Wp.astype(ml_dtypes.bfloat16)

    # 128x256 multiplicative causal triangle (duplicated for the head pair):
    # 1 where key (row) <= query (col) else 0
    k_i = np.arange(128)[:, None]
    q_i = np.arange(128)[None, :]
    cm1 = (k_i <= q_i).astype(np.float32)
    cm = np.concatenate([cm1, cm1], axis=1).astype(ml_dtypes.bfloat16)

    in_maps = []
    for c in range(NCORES):
        b, r = divmod(c, 4)
        hs = slice(256 * r, 256 * (r + 1))
        w_qk = np.concatenate([wq[:, hs], wk[:, hs]], axis=1)
        bqk_t = np.concatenate([bq[hs], bk[hs]]).reshape(4, 128).T.copy()
        zs = np.zeros((128, 2), np.float32)
        zs[:, b] = 1.0
        in_maps.append(dict(
            zsel=zs,
            xT=np.ascontiguousarray(x[b].T).astype(ml_dtypes.bfloat16),
            w_qk=w_qk.astype(ml_dtypes.bfloat16),
            w_v=wv[:, hs].astype(ml_dtypes.bfloat16),
            w_p=wp_bf,
            bqk=bqk_t.astype(np.float32),
            beff=beff,
            cmask=cm,
        ))
    return in_maps


def unshard(results):
    full = np.zeros((B, S, D), np.float32)
    for c in range(NCORES):
        b, r = divmod(c, 4)
        full[b, 512 * r:512 * (r + 1)] = results[c]["out"]
    return full


_NC = None


def kernel(**inputs):
    global _NC
    if _NC is None:
        _NC = build_nc()
    in_maps = shard_inputs(**inputs)
    res = run_bass_kernel_spmd(_NC, in_maps, core_ids=list(range(NCORES)))
    return unshard(res.results)


if __name__ == "__main__":
    import jax
    with jax.default_device(jax.devices("cpu")[0]):
        import reference
        inputs = {k: np.asarray(v) for k, v in reference.setup_inputs().items()}
        expected = np.asarray(reference.reference(**inputs))
    actual = kernel(**inputs)
    err = np.abs(actual - expected)
    print("max abs err:", err.max(), "rel:", err.max() / np.abs(expected).max())


# revision 19
# speedup vs baseline: 1.0480x; 1.0480x over previous
"""Trainium2 Bass kernel for GPT-2 style attention block (B=2, S=2048, D=1024, H=16).

Sharding (8 cores): data-parallel over batch (2) x tensor-parallel over heads (4 per
core). Each core: QKV projection for its 4 heads over the full sequence, full-seq
causal attention for two head PAIRS (scores matmuls K=64 row-tiled so both heads of
a pair run concurrently in the PE array; softmax reduction folded into the PV matmul
via a ones-column in V), then an
AllToAll over all 8 cores converts head-sharding to sequence-sharding so c_proj runs
reduction-free (two-phase accumulation so the second collective overlaps c_proj's
first half). The batch-select is folded into the sender-side normalize so receivers
just add the two AllToAll halves. Host only concatenates per-core outputs.

Compute dtype bf16 (fp32 PSUM accumulation); fp8 was tried for the PV path and
rejected: post-softmax quantization noise lands ~3.6% relative on the output.
"""
import sys
sys.path.insert(0, '/opt/trn_rl_repo')

import numpy as np
import ml_dtypes

import concourse.bass as bass
import concourse.mybir as mybir
import concourse.tile as tile
from concourse import bacc
from concourse.bass_utils import run_bass_kernel_spmd

B, S, D = 2, 2048, 1024
H, HD = 16, 64
NCORES = 8
HPC = H // 4          # heads per core = 4

F32 = mybir.dt.float32
BF16 = mybir.dt.bfloat16
FP8 = mybir.dt.float8e4
DR = mybir.MatmulPerfMode.DoubleRow
ADD = mybir.AluOpType.add
MULT = mybir.AluOpType.mult
BYPASS = mybir.AluOpType.bypass
EXP = mybir.ActivationFunctionType.Exp


def _emit(nc, tc):
    xT = nc.dram_tensor("xT", [D, S], BF16, kind="ExternalInput").ap()
    w_qk = nc.dram_tensor("w_qk", [D, 512], BF16, kind="ExternalInput").ap()
    w_v = nc.dram_tensor("w_v", [D, 256], BF16, kind="ExternalInput").ap()
    w_p = nc.dram_tensor("w_p", [D, D], BF16, kind="ExternalInput").ap()
    bqk = nc.dram_tensor("bqk", [128, 4], F32, kind="ExternalInput").ap()
    zsel = nc.dram_tensor("zsel", [128, 2], F32, kind="ExternalInput").ap()
    beff = nc.dram_tensor("beff", [128, D], F32, kind="ExternalInput").ap()
    cmask = nc.dram_tensor("cmask", [128, 256], BF16, kind="ExternalInput").ap()
    out = nc.dram_tensor("out", [512, D], F32, kind="ExternalOutput").ap()

    a2a_in = [nc.dram_tensor(f"a2a_in{u}", [8, 128, 512], BF16) for u in range(2)]
    a2a_out = [nc.dram_tensor(f"a2a_out{u}", [8, 128, 512], BF16) for u in range(2)]

    from contextlib import ExitStack
    ctx = ExitStack()
    cst = ctx.enter_context(tc.tile_pool(name="cst", bufs=1))
    pw = ctx.enter_context(tc.tile_pool(name="pw", bufs=4, space="PSUM"))
    psc = ctx.enter_context(tc.tile_pool(name="psc", bufs=2, space="PSUM"))
    sb = ctx.enter_context(tc.tile_pool(name="sb", bufs=4))
    sb2 = ctx.enter_context(tc.tile_pool(name="sb2", bufs=2))

    # ---- constants + PE/ACT warm-up (no DMA deps) ----
    junk = cst.tile([128, 512], BF16)
    nc.vector.memset(junk[:], 0.125)
    jexp = cst.tile([128, 8], F32)
    nc.scalar.activation(out=jexp[:], in_=junk[:, 0:8], func=EXP)  # exp table preload
    ones_sb = cst.tile([1, 64], BF16)
    nc.vector.memset(ones_sb[:], 1.0)
    # full-K warm-up matmuls: real array activity so HAM unthrottles early
    for w in range(8):
        warm_ps = pw.tile([128, 512], F32, tag="w", name=f"warm{w}")
        nc.tensor.matmul(warm_ps[:], junk[:, 0:128], junk[:],
                         start=True, stop=True)

    # ---- resident SBUF loads, ordered so qk_proj can start ASAP ----
    xT_sb = cst.tile([128, 8, S], BF16)
    wqk_sb = cst.tile([128, 8, 512], BF16)
    for k in range(8):
        nc.sync.dma_start(wqk_sb[:, k], w_qk.rearrange("(k p) n -> p k n", p=128)[:, k])
        nc.sync.dma_start(xT_sb[:, k], xT.rearrange("(k p) n -> p k n", p=128)[:, k])
    wv_sb = cst.tile([128, 8, 256], BF16)
    nc.sync.dma_start(wv_sb[:], w_v.rearrange("(k p) n -> p k n", p=128))
    bqk_sb = cst.tile([128, 4], F32)
    nc.sync.dma_start(bqk_sb[:], bqk)
    zsel_sb = cst.tile([128, 2], F32)
    nc.sync.dma_start(zsel_sb[:], zsel)
    cm_sb = cst.tile([128, 256], BF16)
    nc.sync.dma_start(cm_sb[:], cmask)
    beff_sb = cst.tile([128, D], F32)
    nc.sync.dma_start(beff_sb[:], beff)
    wp_sb = cst.tile([128, 8, D], BF16)
    nc.sync.dma_start(wp_sb[:], w_p.rearrange("(k p) n -> p k n", p=128))

    # qkT [512, 2048]: slices 0/1 = q^T pairs 0/1 (2 heads x 64 each, q prescaled
    # 1/8), slices 2/3 = k^T pairs 0/1. Head pair layout: even head partitions
    # 0-63, odd head partitions 64-127.
    qkT_sb = cst.tile([128, 4, S], BF16)

    def qk_proj(m):
        ps = {qt: pw.tile([128, 512], F32, tag="w", name=f"qk{m}_{qt}")
              for qt in range(4)}
        for k in range(8):
            for qt in range(4):
                nc.tensor.matmul(
                    ps[qt][:], wqk_sb[:, k, m * 128:(m + 1) * 128],
                    xT_sb[:, k, qt * 512:(qt + 1) * 512],
                    start=(k == 0), stop=(k == 7))
        for qt in range(4):
            nc.vector.tensor_scalar(
                out=qkT_sb[:, m, qt * 512:(qt + 1) * 512], in0=ps[qt][:],
                scalar1=bqk_sb[:, m:m + 1], scalar2=None, op0=ADD)

    # V with interleaved ones column: V_sb [128, 16, 4*65]
    V_sb = cst.tile([128, 16, HPC * 65], BF16)

    def v_ones():
        nc.vector.memset(
            V_sb[:].rearrange("p m (h c) -> p m h c", c=65)[:, :, :, 64:65], 1.0)

    def v_piece(m):
        ps = pw.tile([128, 512], F32, tag="w", name=f"v{m}")
        for k in range(8):
            nc.tensor.matmul(
                ps[:, :256], xT_sb[:, k, m * 128:(m + 1) * 128], wv_sb[:, k, :],
                start=(k == 0), stop=(k == 7))
        nc.vector.tensor_copy(
            out=V_sb[:, m].rearrange("p (h c) -> p h c", c=65)[:, :, 0:64],
            in_=ps[:, :256].rearrange("p (h c) -> p h c", c=64))

    proj_sb = cst.tile([128, 8, 512], BF16)  # gathered attnT for my 512 q rows
    acc_sb = cst.tile([128, 4, D], F32)      # c_proj phase-0 partials

    def attend_pair(u, qt):
        # heads A=2u (partitions 0-63), B=2u+1 (partitions 64-127)
        hA, hB = 2 * u, 2 * u + 1
        nkb = 4 * qt + 4
        atA = pw.tile([128, 512], F32, tag="w", name=f"atA{u}_{qt}")
        atB = pw.tile([128, 512], F32, tag="w", name=f"atB{u}_{qt}")
        for kb in range(nkb):
            rel = max(0, kb * 128 - qt * 512)
            sc = psc.tile([128, 1024], F32, tag="sc", name=f"sc{kb}")
            # scores^T pair: K=64 row-tiles (0,0) and (64,0) run concurrently
            nc.tensor.matmul(
                sc[:, rel:512],
                qkT_sb[0:64, 2 + u, kb * 128:(kb + 1) * 128],
                qkT_sb[0:64, u, qt * 512 + rel:(qt + 1) * 512],
                start=True, stop=True)
            nc.tensor.matmul(
                sc[:, 512 + rel:1024],
                qkT_sb[64:128, 2 + u, kb * 128:(kb + 1) * 128],
                qkT_sb[64:128, u, qt * 512 + rel:(qt + 1) * 512],
                start=True, stop=True)
            pt = sb.tile([128, 1024], BF16, tag="pt", name=f"pt{kb}")
            sc3 = sc[:].rearrange("p (g c) -> p g c", g=2)[:, :, rel:512]
            pt3 = pt[:].rearrange("p (g c) -> p g c", g=2)[:, :, rel:512]
            nc.scalar.activation(out=pt3, in_=sc3, func=EXP)
            if kb >= 4 * qt:  # diagonal block: zero the masked triangle
                ptm = pt[:].rearrange("p (g c) -> p g c", g=2)[:, :, rel:rel + 128]
                nc.vector.tensor_tensor(
                    ptm, ptm, cm_sb[:].rearrange("p (g c) -> p g c", g=2), MULT)
            nc.tensor.matmul(
                atA[0:65, rel:512], V_sb[:, kb, hA * 65:(hA + 1) * 65],
                pt[:, rel:512], start=(kb == 0), stop=(kb == nkb - 1))
            nc.tensor.matmul(
                atB[0:65, rel:512], V_sb[:, kb, hB * 65:(hB + 1) * 65],
                pt[:, 512 + rel:1024], start=(kb == 0), stop=(kb == nkb - 1))
        # unnormalized attn pair -> SBUF, one reciprocal-broadcast for both heads,
        # then batch-select (zsel) folded into the normalize: two send variants
        stg = sb2.tile([128, 512], BF16, tag="stg")
        nc.vector.tensor_copy(out=stg[0:64, :], in_=atA[0:64, :])
        nc.vector.tensor_copy(out=stg[64:128, :], in_=atB[0:64, :])
        den2 = sb2.tile([1, 1024], F32, tag="den2")
        nc.vector.tensor_copy(out=den2[0:1, 0:512], in_=atA[64:65, :])
        nc.vector.tensor_copy(out=den2[0:1, 512:1024], in_=atB[64:65, :])
        rec2 = sb2.tile([1, 1024], F32, tag="rec2")
        nc.vector.reciprocal_approx_fast(rec2[:], den2[:])
        rec2b = sb2.tile([1, 1024], BF16, tag="rec2b")
        nc.vector.tensor_copy(out=rec2b[:], in_=rec2[:])
        bc = pw.tile([128, 512], F32, tag="w", name=f"bc{u}_{qt}")
        nc.tensor.matmul(bc[0:64, :], ones_sb[:], rec2b[0:1, 0:512],
                         start=True, stop=True)
        nc.tensor.matmul(bc[64:128, :], ones_sb[:], rec2b[0:1, 512:1024],
                         start=True, stop=True)
        for half, tag in ((0, "sA"), (1, "sB")):
            snd = sb2.tile([128, 512], BF16, tag=tag)
            nc.vector.scalar_tensor_tensor(
                out=snd[:], in0=stg[:], scalar=zsel_sb[:, half:half + 1],
                in1=bc[:], op0=MULT, op1=MULT)
            nc.sync.dma_start(a2a_in[u].ap()[4 * half + qt], snd[:])

    def a2a_go(u):
        nc.gpsimd.collective_compute(
            "AllToAll", BYPASS, replica_groups=[list(range(NCORES))],
            ins=[a2a_in[u].ap().opt()], outs=[a2a_out[u].ap().opt()])

    def a2a_recv(u, eng):
        # eng=gpsimd for the mid-kernel recv: a blocked wait in the Vector FIFO
        # would stall every attend op queued behind it (engines are strict FIFO)
        t0 = sb2.tile([128, 4, 512], BF16, tag="t0")
        t1 = sb2.tile([128, 4, 512], BF16, tag="t1")
        nc.sync.dma_start(t0[:], a2a_out[u].ap()[0:4].rearrange("g p n -> p g n"))
        nc.sync.dma_start(t1[:], a2a_out[u].ap()[4:8].rearrange("g p n -> p g n"))
        dst = proj_sb[:].rearrange("p (g t) n -> p g t n", t=2)[:, :, u, :]
        eng.tensor_tensor(dst, t0[:], t1[:], ADD)

    def c_proj_phase0(ms):
        # unit-0 chunks (k = 0,2,4,6) -> acc_sb (+ beff), frees PSUM per step
        for m in ms:
            for n in range(2):
                ps = pw.tile([128, 512], F32, tag="w", name=f"p0_{m}_{n}")
                for ki, k in enumerate((0, 2, 4, 6)):
                    nc.tensor.matmul(
                        ps[:], proj_sb[:, k, m * 128:(m + 1) * 128],
                        wp_sb[:, k, n * 512:(n + 1) * 512],
                        start=(ki == 0), stop=(ki == 3))
                nc.vector.tensor_tensor(
                    acc_sb[:, m, n * 512:(n + 1) * 512], ps[:],
                    beff_sb[:, n * 512:(n + 1) * 512], ADD)

    def c_proj_phase1(ms):
        for m in ms:
            out_sb = sb2.tile([128, D], F32, tag="out")
            for n in range(2):
                ps = pw.tile([128, 512], F32, tag="w", name=f"p1_{m}_{n}")
                for ki, k in enumerate((1, 3, 5, 7)):
                    nc.tensor.matmul(
                        ps[:], proj_sb[:, k, m * 128:(m + 1) * 128],
                        wp_sb[:, k, n * 512:(n + 1) * 512],
                        start=(ki == 0), stop=(ki == 3))
                nc.vector.tensor_tensor(
                    out_sb[:, n * 512:(n + 1) * 512], ps[:],
                    acc_sb[:, m, n * 512:(n + 1) * 512], ADD)
            nc.sync.dma_start(out[m * 128:(m + 1) * 128, :], out_sb[:])

    # ---- emission order tuned for overlap ----
    qk_proj(0)
    qk_proj(2)
    v_ones()
    for m in range(4):
        v_piece(m)
    attend_pair(0, 0)
    for m in range(4, 8):
        v_piece(m)
    attend_pair(0, 1)
    for m in range(8, 12):
        v_piece(m)
    attend_pair(0, 2)
    for m in range(12, 16):
        v_piece(m)
    attend_pair(0, 3)
    a2a_go(0)
    qk_proj(1)
    qk_proj(3)
    attend_pair(1, 0)
    a2a_recv(0, nc.gpsimd)
    attend_pair(1, 1)
    c_proj_phase0((0, 1))
    attend_pair(1, 2)
    c_proj_phase0((2, 3))
    attend_pair(1, 3)
    a2a_go(1)
    for w in range(24):
        warm_ps = pw.tile([128, 512], F32, tag="w", name=f"bridge{w}")
        nc.tensor.matmul(warm_ps[:], junk[:, 0:128], junk[:],
                         start=True, stop=True)
    a2a_recv(1, nc.vector)
    c_proj_phase1((0, 1, 2, 3))

    ctx.close()


def build_nc():
    nc = bacc.Bacc("TRN2", target_bir_lowering=False, debug=False, num_devices=NCORES)
    with tile.TileContext(nc) as tc:
        _emit(nc, tc)
    nc.compile()
    return nc


def shard_inputs(hidden_states, c_attn_w, c_attn_b, c_proj_w, c_proj_b):
    x = np.asarray(hidden_states, np.float32)
    W = np.asarray(c_attn_w, np.float32)
    bqkv = np.asarray(c_attn_b, np.float32)
    Wp = np.asarray(c_proj_w, np.float32)
    bp = np.asarray(c_proj_b, np.float32)

    wq, wk, wv = W[:, :D] * 0.125, W[:, D:2 * D], W[:, 2 * D:]
    bq, bk, bv = bqkv[:D] * 0.125, bqkv[D:2 * D], bqkv[2 * D:]
    beff = np.broadcast_to(bp + bv @ Wp, (128, D)).astype(np.float32).copy()
    wp_bf = Wp.astype(ml_dtypes.bfloat16)

    # 128x256 multiplicative causal triangle (duplicated for the head pair):
    # 1 where key (row) <= query (col) else 0
    k_i = np.arange(128)[:, None]
    q_i = np.arange(128)[None, :]
    cm1 = (k_i <= q_i).astype(np.float32)
    cm = np.concatenate([cm1, cm1], axis=1).astype(ml_dtypes.bfloat16)

    in_maps = []
    for c in range(NCORES):
        b, r = divmod(c, 4)
        hs = slice(256 * r, 256 * (r + 1))
        w_qk = np.concatenate([wq[:, hs], wk[:, hs]], axis=1)
        bqk_t = np.concatenate([bq[hs], bk[hs]]).reshape(4, 128).T.copy()
        zs = np.zeros((128, 2), np.float32)
        zs[:, b] = 1.0
        in_maps.append(dict(
            zsel=zs,
            xT=np.ascontiguousarray(x[b].T).astype(ml_dtypes.bfloat16),
            w_qk=w_qk.astype(ml_dtypes.bfloat16),
            w_v=wv[:, hs].astype(ml_dtypes.bfloat16),
            w_p=wp_bf,
            bqk=bqk_t.astype(np.float32),
            beff=beff,
            cmask=cm,
        ))
    return in_maps


def unshard(results):
    full = np.zeros((B, S, D), np.float32)
    for c in range(NCORES):
        b, r = divmod(c, 4)
        full[b, 512 * r:512 * (r + 1)] = results[c]["out"]
    return full


_NC = None


def kernel(**inputs):
    global _NC
    if _NC is None:
        _NC = build_nc()
    in_maps = shard_inputs(**inputs)
    res = run_bass_kernel_spmd(_NC, in_maps, core_ids=list(range(NCORES)))
    return unshard(res.results)


if __name__ == "__main__":
    import jax
    with jax.default_device(jax.devices("cpu")[0]):
        import reference
        inputs = {k: np.asarray(v) for k, v in reference.setup_inputs().items()}
        expected = np.asarray(reference.reference(**inputs))
    actual = kernel(**inputs)
    err = np.abs(actual - expected)
    print("max abs err:", err.max(), "rel:", err.max() / np.abs(expected).max())
